# revision 29
# baseline (speedup 1.0000x reference)
"""NsNet2 single-step (fc1 + 2x GRU cell + 3x FC) Trainium2 kernel.

Strategy:
  - Pure data parallel: batch B=32768 sharded as 4096 rows per NeuronCore (8 cores).
  - Feature-major layout on chip: activations live as [feat, batch]; host
    transposes inputs/outputs (free; off the HW critical path).
  - All matmuls fp8(e4m3) with DoubleRow perf mode (2 K-chunks per pass) and
    fp32 PSUM accumulation; biases fused into ScalarE activation / VectorE ops.
  - fc1 folded into the GRU1 input-gate weights on the host.
  - z,r gates: input-side and hidden-side summed in one PSUM. GRU1 contracts a
    host-packed [x|h1] operand (657 rows -> 6 chunks, 3 DR passes); GRU2 uses
    two accumulation groups (g1-aligned + h2-aligned, 2+2 DR passes) to avoid
    an on-chip concat.
  - Software pipelining across the 8 batch tiles per core: PE work is emitted
    at matmul-group granularity as GRU1(t)'s n-units interleaved with FC(t-1),
    then GRU2(t)'s zr, then GRU2(t)'s n-units interleaved with GRU1(t+1)'s zr,
    so the tensor engine always has independent work while gate chains drain.
  - PE HAM warmup (dummy matmuls during the initial weight DMA) so real matmuls
    start at the full 2.4 GHz clock.
  - Blend h' = z*h + (1-z)*n: z*h and (1-z) precomputed on GpSimd off the
    critical path; only u*n and the final add (VectorE) trail the tanh.
"""

import os
import sys

import numpy as np
import ml_dtypes

sys.path.insert(0, "/opt/trn_rl_repo")

import concourse.bacc as bacc
import concourse.bass as bass
import concourse.mybir as mybir
import concourse.tile as tile
from concourse.bass import ts
from concourse.bass_utils import run_bass_kernel_spmd

BF16 = ml_dtypes.bfloat16
FP8 = ml_dtypes.float8_e4m3

B, F, H, FF = 32768, 257, 400, 600
NCORES = 8
BPC = B // NCORES          # 4096 batch rows per core
Hp, FFp, Fp = 512, 640, 384  # padded feature dims
XH1 = 769                  # [x(257) | h1(400) | pad(112)] rows
ZRM = 800                  # [z(0:384) | r(0:384) | z-stub(16) | r-stub(16)] cols
ZRC = 7                    # 6 full M chunks + 32-wide stub chunk
# column permutation: old [z(400)|r(400)] -> new layout above, so r chunks are
# lane-aligned with h chunks (no on-chip realign DMAs)
ZR_PERM = np.zeros(800, dtype=np.int64)
for _g in range(2):
    ZR_PERM[_g * 384 : (_g + 1) * 384] = np.arange(_g * 400, _g * 400 + 384)
ZR_PERM[768:784] = np.arange(384, 400)        # z features 384..399
ZR_PERM[784:800] = np.arange(784, 800)        # r features 384..399
NB = 512                   # matmul free-dim tile (one PSUM bank of fp32)

AF = mybir.ActivationFunctionType
ALU = mybir.AluOpType

# packed bias column layout: name -> (offset, n_chunks)
BIAS_LAYOUT = {}
_off = 0
for _n, _c in (("bzr1", 7), ("bnx1", 4), ("bnh1", 4),
               ("bzr2", 7), ("bnx2", 4), ("bnh2", 4),
               ("bfc2", 5), ("bfc3", 5), ("bfc4", 3)):
    BIAS_LAYOUT[_n] = (_off, _c)
    _off += _c
BIAS_COLS = _off


def _pad2(a, rows, cols):
    out = np.zeros((rows, cols), dtype=np.float64)
    out[: a.shape[0], : a.shape[1]] = a
    return out


def _bias_tile(vec, padded):
    """Pack a [padded] bias vector as [128, padded//128] fp32 (partition-major)."""
    v = np.zeros(padded, dtype=np.float64)
    v[: vec.shape[0]] = vec
    return np.ascontiguousarray(v.reshape(padded // 128, 128).T).astype(np.float32)


def prepare_weights(inp):
    f64 = {k: np.asarray(v, dtype=np.float64) for k, v in inp.items()}
    w = {}

    # fc1 fold for GRU1 input side
    Wx = {}
    bx = {}
    for name in ("z", "r", "n"):
        Wx[name] = (f64[f"Wi{name}1"] @ f64["Wfc1"]).T          # [F, H]
        bx[name] = f64[f"bi{name}1"] + f64[f"Wi{name}1"] @ f64["bfc1"]

    # GRU1 z,r: K-concat [x(257) | h1(400)] -> rows 0..656, M=[z|r]=800
    Wzr1 = np.zeros((768, ZRM), dtype=np.float64)
    for g, name in enumerate(("z", "r")):
        Wzr1[:F, g * H : g * H + H] = Wx[name]
        Wzr1[F : F + H, g * H : g * H + H] = f64[f"Wh{name}1"].T
    w["Wzr1"] = Wzr1[:, ZR_PERM]
    # GRU1 n input side contracts xh chunks 0..3; rows 257.. are h1 -> zero
    w["Wn1x"] = _pad2(Wx["n"], Hp, Hp)
    w["Wn1h"] = _pad2(f64["Whn1"].T, Hp, Hp)

    # GRU2 z,r: two accumulation groups (g1-side, h2-side), each K=512 padded
    Wzr2g = np.zeros((Hp, ZRM), dtype=np.float64)
    Wzr2h = np.zeros((Hp, ZRM), dtype=np.float64)
    for g, name in enumerate(("z", "r")):
        Wzr2g[:H, g * H : g * H + H] = f64[f"Wi{name}2"].T
        Wzr2h[:H, g * H : g * H + H] = f64[f"Wh{name}2"].T
    w["Wzr2g"] = Wzr2g[:, ZR_PERM]
    w["Wzr2h"] = Wzr2h[:, ZR_PERM]
    w["Wn2x"] = _pad2(f64["Win2"].T, Hp, Hp)
    w["Wn2h"] = _pad2(f64["Whn2"].T, Hp, Hp)

    w["Wfc2T"] = _pad2(f64["Wfc2"].T, Hp, FFp)    # [512, 640]
    w["Wfc3T"] = _pad2(f64["Wfc3"].T, FFp, FFp)   # [640, 640]
    w["Wfc4T"] = _pad2(f64["Wfc4"].T, FFp, Fp)    # [640, 384]

    weights = {k: np.ascontiguousarray(v).astype(FP8) for k, v in w.items()}

    parts = [
        ("bzr1", _bias_tile(np.concatenate([bx["z"] + f64["bhz1"],
                                            bx["r"] + f64["bhr1"]])[ZR_PERM],
                            896)),
        ("bnx1", _bias_tile(bx["n"], Hp)),
        ("bnh1", _bias_tile(f64["bhn1"], Hp)),
        ("bzr2", _bias_tile(np.concatenate([f64["biz2"] + f64["bhz2"],
                                            f64["bir2"] + f64["bhr2"]])[ZR_PERM],
                            896)),
        ("bnx2", _bias_tile(f64["bin2"], Hp)),
        ("bnh2", _bias_tile(f64["bhn2"], Hp)),
        ("bfc2", _bias_tile(f64["bfc2"], FFp)),
        ("bfc3", _bias_tile(f64["bfc3"], FFp)),
        ("bfc4", _bias_tile(f64["bfc4"], Fp)),
    ]
    biases = {"biasT": np.concatenate([p[1] for p in parts], axis=1)}
    return weights, biases


def build_nc(nbt=BPC, nb=NB):
    """Build the per-core Bass program. nbt = per-core batch, nb = free-dim tile."""
    nc = bacc.Bacc("TRN2", target_bir_lowering=False, debug=False)
    bf = mybir.dt.bfloat16
    f32 = mybir.dt.float32
    f8 = mybir.dt.float8e4

    xh8 = nc.declare_dram_parameter("xh8", [XH1, nbt], f8, isOutput=False)
    h1T = nc.declare_dram_parameter("h1T", [Hp, nbt], bf, isOutput=False)
    h2T = nc.declare_dram_parameter("h2T", [Hp, nbt], bf, isOutput=False)
    h28 = nc.declare_dram_parameter("h28", [Hp, nbt], f8, isOutput=False)
    wd = {}
    for name, k, m in (
        ("Wzr1", 768, ZRM), ("Wn1x", Hp, Hp), ("Wn1h", Hp, Hp),
        ("Wzr2g", Hp, ZRM), ("Wzr2h", Hp, ZRM),
        ("Wn2x", Hp, Hp), ("Wn2h", Hp, Hp),
        ("Wfc2T", Hp, FFp), ("Wfc3T", FFp, FFp), ("Wfc4T", FFp, Fp),
    ):
        wd[name] = nc.declare_dram_parameter(name, [k, m], f8, isOutput=False)
    biasT_d = nc.declare_dram_parameter("biasT", [128, BIAS_COLS], f32, isOutput=False)
    outT = nc.declare_dram_parameter("outT", [Fp, nbt], bf, isOutput=True)

    n_tiles = nbt // nb
    HC = Hp // 128   # 4
    FC = FFp // 128  # 5
    DR = mybir.MatmulPerfMode.DoubleRow

    with tile.TileContext(nc) as tc:
        with (
            tc.tile_pool(name="wpool", bufs=1) as wpool,
            tc.tile_pool(name="bpool", bufs=1) as bpool,
            tc.tile_pool(name="io", bufs=3) as io,
            tc.tile_pool(name="inp", bufs=3) as inp,
            tc.tile_pool(name="act", bufs=3) as act,
            tc.tile_pool(name="psum", bufs=2, space="PSUM") as psum,
        ):
            # ACT-table warmup off the critical chain.
            warm = bpool.tile([128, 1], f32, tag="warm")
            nc.vector.memset(warm, 0.0)
            nc.scalar.activation(warm, warm, AF.Sigmoid)
            # PE HAM warmup: ~3.4us of dummy matmuls on zeroed SBUF so the PE
            # clock is at 8/8 when the first real matmul's operands land.
            wz8 = bpool.tile([128, 2, 128], f8, tag="wz8")
            nc.vector.memset(wz8, 0.0)
            pw = psum.tile([128, nb], f32, tag="ps_zr")
            for _ in range(32):
                nc.tensor.matmul(pw[:, 0:128], wz8[:, 0, :], wz8[:, 1, :],
                                 start=True, stop=True)

            W, BT = {}, {}

            def load_w(name, eng):
                dram = wd[name]
                k, m = dram.shape
                t = wpool.tile([128, k // 128, m], dram.dtype, tag=name)
                eng.dma_start(out=t, in_=dram.rearrange("(c p) m -> p c m", p=128))
                W[name] = t

            def load_bias():
                biasT = bpool.tile([128, BIAS_COLS], f32, tag="biasT")
                nc.scalar.dma_start(out=biasT, in_=biasT_d[:, :])
                for _n, (_o, _c) in BIAS_LAYOUT.items():
                    BT[_n] = biasT[:, _o : _o + _c]

            xh_zr = xh8[0:768, :].rearrange("(c p) n -> p c n", p=128)
            h1m_al = xh8[257 : 257 + Hp, :].rearrange("(c p) n -> p c n", p=128)
            h1_bl = h1T.rearrange("(c p) n -> p c n", p=128)
            h2_bl = h2T.rearrange("(c p) n -> p c n", p=128)
            h28_al = h28.rearrange("(c p) n -> p c n", p=128)
            outT_r = outT.rearrange("(c p) n -> p c n", p=128)

            INP, ZOP, G2 = {}, {}, {}

            def load_inputs(t):
                sl = ts(t, nb)
                xh = inp.tile([128, 6, nb], f8, tag="xh")
                nc.sync.dma_start(out=xh, in_=xh_zr[:, :, sl])
                h1m = inp.tile([128, HC, nb], f8, tag="h1m")
                nc.sync.dma_start(out=h1m, in_=h1m_al[:, :, sl])
                h1s = inp.tile([128, HC, nb], bf, tag="h1s")
                nc.sync.dma_start(out=h1s, in_=h1_bl[:, :, sl])
                h2s = inp.tile([128, HC, nb], bf, tag="h2s")
                nc.sync.dma_start(out=h2s, in_=h2_bl[:, :, sl])
                h28s = inp.tile([128, HC, nb], f8, tag="h28s")
                nc.sync.dma_start(out=h28s, in_=h28_al[:, :, sl])
                INP[t] = (xh, h1m, h1s, h2s, h28s)

            def mm_acc(ps, parts, col, mw):
                """Accumulate sum_g (W_g.T @ rhs_g) for output cols [col, col+mw)
                into ps. parts: list of (Wtile, kc, rhs_tile); DR on chunk pairs."""
                total = sum((kc + 1) // 2 for _, kc, _ in parts)
                i = 0
                for Wt, kc, rhs in parts:
                    k = 0
                    while k < kc:
                        if k + 2 <= kc:
                            nc.tensor.matmul(
                                ps, Wt[:, k : k + 2, col : col + mw],
                                rhs[:, k : k + 2, :],
                                start=(i == 0), stop=(i == total - 1),
                                perf_mode=DR)
                            k += 2
                        else:
                            nc.tensor.matmul(
                                ps, Wt[:, k, col : col + mw], rhs[:, k, :],
                                start=(i == 0), stop=(i == total - 1))
                            k += 1
                        i += 1

            def make_gru(zr_parts, nx_parts, nh_parts, hs, bzr, bnx, bnh,
                         out_chunk, pre_z0=None, add_eng=None):
                """Unit closures for one GRU step. h' = z*h + (1-z)*n.
                zro chunks: 0-2 = z[0:384], 3-5 = r[0:384] (lane-aligned with
                h chunks), 6 = [z-stub(16) | r-stub(16)]."""
                st = {}

                def z_unit(c):
                    def run():
                        if c == 0:
                            if pre_z0 is not None:
                                pre_z0()
                            st["zro"] = act.tile([128, ZRC, nb], bf, tag="zro", name="zro")
                        zro = st["zro"]
                        mw = min(128, ZRM - c * 128)
                        ps = psum.tile([128, nb], f32, tag="ps_zr")
                        mm_acc(ps[:mw, :], zr_parts, c * 128, mw)
                        nc.scalar.activation(zro[:mw, c, :], ps[:mw, :],
                                             AF.Sigmoid, bias=bzr[:mw, c : c + 1])
                        if c == 6:
                            # r stub: parts 16:32 of chunk 6 -> parts 0:16
                            r_st = act.tile([128, nb], bf, tag="r_st")
                            nc.sync.dma_start(out=r_st[0:16, :],
                                              in_=zro[16:32, 6, :])
                            st["r_st"] = r_st
                            zh = act.tile([128, HC, nb], bf, tag="zh")
                            u = act.tile([128, HC, nb], bf, tag="u")
                            nc.gpsimd.tensor_mul(zh[:, 0:3, :], zro[:, 0:3, :],
                                                 hs[:, 0:3, :])
                            nc.gpsimd.tensor_mul(zh[0:16, 3, :], zro[0:16, 6, :],
                                                 hs[0:16, 3, :])
                            nc.gpsimd.tensor_scalar(u[:, 0:3, :], zro[:, 0:3, :],
                                                    -1.0, 1.0, op0=ALU.mult,
                                                    op1=ALU.add)
                            nc.gpsimd.tensor_scalar(u[0:16, 3, :],
                                                    zro[0:16, 6, :], -1.0, 1.0,
                                                    op0=ALU.mult, op1=ALU.add)
                            st["zh"], st["u"] = zh, u
                    return run

                def n_unit(m):
                    def run():
                        zro, r_st = st["zro"], st["r_st"]
                        zh, u = st["zh"], st["u"]
                        pz = 128 if m < 3 else 16
                        col = m * 128
                        psh = psum.tile([128, nb], f32, tag="ps_nh")
                        mm_acc(psh, nh_parts, col, 128)
                        # alternate psx between tags: npre (its consumer) is the
                        # slowest chain op, and a 2-buf rotation would stall
                        # psx(m) on npre(m-2); borrowing ps_fc doubles the depth
                        psx = psum.tile([128, nb], f32,
                                        tag="ps_nx" if m % 2 == 0 else "ps_fc")
                        mm_acc(psx, nx_parts, col, 128)
                        r_ap = zro[:pz, 3 + m, :] if m < 3 else r_st[0:16, :]
                        rhn = act.tile([128, nb], bf, tag="rhn")
                        nc.vector.scalar_tensor_tensor(
                            rhn[:pz, :], psh[:pz, :], bnh[:pz, m : m + 1],
                            r_ap, op0=ALU.add, op1=ALU.mult)
                        npre = act.tile([128, nb], f32, tag="npre")
                        nc.vector.scalar_tensor_tensor(
                            npre[:pz, :], psx[:pz, :], bnx[:pz, m : m + 1],
                            rhn[:pz, :], op0=ALU.add, op1=ALU.add)
                        n_t = act.tile([128, nb], bf, tag="n_t")
                        nc.scalar.activation(n_t[:pz, :], npre[:pz, :], AF.Tanh)
                        un = act.tile([128, nb], bf, tag="un")
                        nc.vector.tensor_mul(un[:pz, :], u[:pz, m, :], n_t[:pz, :])
                        out_ap = out_chunk(m)
                        p = min(out_ap.shape[0], pz)
                        (add_eng or nc.vector).tensor_add(
                            out_ap[:p, :] if out_ap.shape[0] > p else out_ap,
                            zh[:p, m, :], un[:p, :])
                    return run

                return [z_unit(c) for c in range(ZRC)], \
                       [n_unit(m) for m in range(HC)]

            def make_gru1(t):
                xh, h1m, h1s, h2s, h28s = INP[t]
                zop = io.tile([128, HC, nb], f8, tag="zop")
                ZOP[t] = zop

                def pre_z0():
                    nc.gpsimd.memset(zop[:, 3, :], 0.0)

                def g1_out(m):
                    return zop[:, m, :] if m < 3 else zop[0:16, 3, :]

                return make_gru([(W["Wzr1"], 6, xh)],
                                [(W["Wn1x"], HC, xh)],
                                [(W["Wn1h"], HC, h1m)],
                                h1s, BT["bzr1"], BT["bnx1"], BT["bnh1"],
                                g1_out, pre_z0)

            def make_gru2(t):
                xh, h1m, h1s, h2s, h28s = INP[t]
                zop = ZOP[t]
                g2 = io.tile([128, HC, nb], f8, tag="g2")
                G2[t] = g2

                def pre_z0():
                    nc.gpsimd.memset(g2[:, 3, :], 0.0)

                return make_gru([(W["Wzr2g"], HC, zop), (W["Wzr2h"], HC, h28s)],
                                [(W["Wn2x"], HC, zop)],
                                [(W["Wn2h"], HC, h28s)],
                                h2s, BT["bzr2"], BT["bnx2"], BT["bnh2"],
                                lambda m: g2[:, m, :] if m < 3 else g2[0:16, 3, :],
                                pre_z0)

            def make_fc(t):
                """Unit closures for fc2/fc3/fc4 of tile t. f3 alternates its
                PSUM between ps_fc and ps_nh so back-to-back groups never wait
                on the relu of the group two before."""
                g2 = G2[t]
                st = {}

                def f2_unit(m):
                    def run():
                        if m == 0:
                            st["f2"] = io.tile([128, FC, nb], f8, tag="f2", name="f2")
                        tag = "ps_fc" if m % 2 == 0 else "ps_zr"
                        ps = psum.tile([128, nb], f32, tag=tag)
                        mm_acc(ps, [(W["Wfc2T"], HC, g2)], m * 128, 128)
                        nc.scalar.activation(st["f2"][:, m, :], ps, AF.Relu,
                                             bias=BT["bfc2"][:, m : m + 1])
                    return run

                def f3_unit(m):
                    def run():
                        if m == 0:
                            st["f3"] = io.tile([128, FC, nb], f8, tag="f3", name="f3")
                        tag = "ps_fc" if m % 2 == 0 else "ps_nh"
                        ps = psum.tile([128, nb], f32, tag=tag)
                        mm_acc(ps, [(W["Wfc3T"], FC, st["f2"])], m * 128, 128)
                        nc.vector.tensor_scalar(st["f3"][:, m, :], ps,
                                                BT["bfc3"][:, m : m + 1], 0.0,
                                                op0=ALU.add, op1=ALU.max)
                    return run

                def f4_unit(m):
                    def run():
                        if m == 0:
                            st["o"] = io.tile([128, Fp // 128, nb], bf, tag="o", name="o")
                        ps = psum.tile([128, nb], f32, tag="ps_fc")
                        mm_acc(ps, [(W["Wfc4T"], FC, st["f3"])], m * 128, 128)
                        nc.scalar.activation(st["o"][:, m, :], ps, AF.Sigmoid,
                                             bias=BT["bfc4"][:, m : m + 1])
                    return run

                def out_unit():
                    nc.sync.dma_start(out=outT_r[:, :, ts(t, nb)], in_=st["o"])

                return ([f2_unit(m) for m in range(FC)],
                        [f3_unit(m) for m in range(FC)],
                        [f4_unit(m) for m in range(Fp // 128)], out_unit)

            # startup: interleave tile-0 inputs with the weights needed first
            sl0 = ts(0, nb)
            load_w("Wzr1", nc.sync)
            xh0 = inp.tile([128, 6, nb], f8, tag="xh")
            nc.sync.dma_start(out=xh0, in_=xh_zr[:, :, sl0])
            load_bias()
            h1m0 = inp.tile([128, HC, nb], f8, tag="h1m")
            nc.sync.dma_start(out=h1m0, in_=h1m_al[:, :, sl0])
            load_w("Wn1x", nc.scalar)
            load_w("Wn1h", nc.scalar)
            h1s0 = inp.tile([128, HC, nb], bf, tag="h1s")
            nc.sync.dma_start(out=h1s0, in_=h1_bl[:, :, sl0])
            h2s0 = inp.tile([128, HC, nb], bf, tag="h2s")
            nc.sync.dma_start(out=h2s0, in_=h2_bl[:, :, sl0])
            h28s0 = inp.tile([128, HC, nb], f8, tag="h28s")
            nc.sync.dma_start(out=h28s0, in_=h28_al[:, :, sl0])
            INP[0] = (xh0, h1m0, h1s0, h2s0, h28s0)
            for name in ("Wzr2g", "Wzr2h", "Wn2x", "Wn2h",
                         "Wfc2T", "Wfc3T", "Wfc4T"):
                load_w(name, nc.scalar)

            # Steady-state PE emission per iteration t (units for tiles
            # t-1 / t / t+1): GRU1(t)'s n units interleaved with FC(t-1)'s
            # fc2, then fc3/fc4, then GRU2(t)'s zr, then GRU2(t)'s n units
            # interleaved with GRU1(t+1)'s zr — so the tensor engine always
            # has independent work while each gate chain drains.
            Z1, N1 = make_gru1(0)
            for z in Z1:
                z()
            Z1_done = False  # True when tile t+1's zr units were pre-emitted
            for t in range(n_tiles):
                if t + 1 < n_tiles:
                    load_inputs(t + 1)
                F2, F3, F4, FOUT = make_fc(t - 1) if t >= 1 else \
                    ([None] * FC, [None] * FC, [None] * 3, None)

                def emit(u):
                    if u is not None:
                        u()

                if t == 0:
                    # no FC(t-1) filler on the first tile: interleave the next
                    # tile's zr units into the n-chain instead
                    Z1n, N1n = make_gru1(1)
                    N1[0]()
                    N1[1]()
                    Z1n[0]()
                    Z1n[1]()
                    N1[2]()
                    Z1n[2]()
                    Z1n[3]()
                    N1[3]()
                    Z1n[4]()
                    Z1n[5]()
                    Z1n[6]()
                    Z1_done = True
                else:
                    N1[0]()
                    N1[1]()
                    emit(F2[0])
                    N1[2]()
                    emit(F2[1])
                    emit(F2[2])
                    N1[3]()
                    emit(F2[3])
                    emit(F2[4])
                for u in F3:
                    emit(u)
                Z2, N2 = make_gru2(t)
                Z2[0]()
                emit(F4[0])
                Z2[1]()
                emit(F4[1])
                Z2[2]()
                emit(F4[2])
                emit(FOUT)
                Z2[3]()
                Z2[4]()
                Z2[5]()
                Z2[6]()
                if t + 1 < n_tiles:
                    if not Z1_done:
                        Z1n, N1n = make_gru1(t + 1)
                        N2[0]()
                        Z1n[0]()
                        Z1n[1]()
                        N2[1]()
                        Z1n[2]()
                        Z1n[3]()
                        N2[2]()
                        Z1n[4]()
                        Z1n[5]()
                        N2[3]()
                        Z1n[6]()
                    else:
                        for u in N2:
                            u()
                        Z1_done = False
                    N1 = N1n
                else:
                    for u in N2:
                        u()
            F2, F3, F4, FOUT = make_fc(n_tiles - 1)
            for u in F2 + F3 + F4:
                u()
            FOUT()

    nc.compile()
    return nc


def _shard_inputs(inp, weights, biases):
    x = np.asarray(inp["x"], dtype=np.float32)
    h1 = np.asarray(inp["h1"], dtype=np.float32)
    h2 = np.asarray(inp["h2"], dtype=np.float32)

    xh8 = np.zeros((NCORES, XH1, BPC), dtype=FP8)    # matmul operand [x|h1]
    h1T = np.zeros((NCORES, Hp, BPC), dtype=BF16)    # blend h1
    h2T = np.zeros((NCORES, Hp, BPC), dtype=BF16)    # blend h2
    h28 = np.zeros((NCORES, Hp, BPC), dtype=FP8)     # matmul h2
    for i in range(NCORES):
        sl = slice(i * BPC, (i + 1) * BPC)
        xh8[i, :F] = x[sl].T.astype(FP8)
        xh8[i, F : F + H] = h1[sl].T.astype(FP8)
        h1T[i, :H] = h1[sl].T.astype(BF16)
        h2T[i, :H] = h2[sl].T.astype(BF16)
        h28[i, :H] = h2[sl].T.astype(FP8)

    in_maps = []
    for i in range(NCORES):
        m = {"xh8": xh8[i], "h1T": h1T[i], "h2T": h2T[i], "h28": h28[i]}
        m.update(weights)
        m.update(biases)
        in_maps.append(m)
    return in_maps


def _run(inp, trace=False):
    weights, biases = prepare_weights(inp)
    nc = build_nc()
    in_maps = _shard_inputs(inp, weights, biases)
    res = run_bass_kernel_spmd(nc, in_maps, list(range(NCORES)), trace=trace)
    out = np.empty((B, F), dtype=np.float32)
    for i in range(NCORES):
        out[i * BPC : (i + 1) * BPC] = (
            np.asarray(res.results[i]["outT"][:F]).astype(np.float32).T
        )
    return out, res


def kernel(**inputs) -> np.ndarray:
    out, _ = _run(inputs, trace=False)
    return out


# revision 30
# speedup vs baseline: 1.0207x; 1.0207x over previous
"""NsNet2 single-step (fc1 + 2x GRU cell + 3x FC) Trainium2 kernel.

Strategy:
  - Pure data parallel: batch B=32768 sharded as 4096 rows per NeuronCore (8 cores).
  - Feature-major layout on chip: activations live as [feat, batch]; host
    transposes inputs/outputs (free; off the HW critical path).
  - All matmuls fp8(e4m3) with DoubleRow perf mode (2 K-chunks per pass) and
    fp32 PSUM accumulation; biases fused into ScalarE activation / VectorE ops.
  - fc1 folded into the GRU1 input-gate weights on the host.
  - z,r gates: input-side and hidden-side summed in one PSUM. GRU1 contracts a
    host-packed [x|h1] operand (657 rows -> 6 chunks, 3 DR passes); GRU2 uses
    two accumulation groups (g1-aligned + h2-aligned, 2+2 DR passes) to avoid
    an on-chip concat.
  - Software pipelining across the 8 batch tiles per core: PE work is emitted
    at matmul-group granularity as GRU1(t)'s n-units interleaved with FC(t-1),
    then GRU2(t)'s zr, then GRU2(t)'s n-units interleaved with GRU1(t+1)'s zr,
    so the tensor engine always has independent work while gate chains drain.
  - PE HAM warmup (dummy matmuls during the initial weight DMA) so real matmuls
    start at the full 2.4 GHz clock.
  - Blend h' = z*h + (1-z)*n: z*h and (1-z) precomputed on GpSimd off the
    critical path; only u*n and the final add (VectorE) trail the tanh.
"""

import os
import sys

import numpy as np
import ml_dtypes

sys.path.insert(0, "/opt/trn_rl_repo")

import concourse.bacc as bacc
import concourse.bass as bass
import concourse.mybir as mybir
import concourse.tile as tile
from concourse.bass import ts
from concourse.bass_utils import run_bass_kernel_spmd

BF16 = ml_dtypes.bfloat16
FP8 = ml_dtypes.float8_e4m3

B, F, H, FF = 32768, 257, 400, 600
NCORES = 8
BPC = B // NCORES          # 4096 batch rows per core
Hp, FFp, Fp = 512, 640, 384  # padded feature dims
XH1 = 769                  # [x(257) | h1(400) | pad(112)] rows
ZRM = 800                  # [z(0:384) | r(0:384) | z-stub(16) | r-stub(16)] cols
ZRC = 7                    # 6 full M chunks + 32-wide stub chunk
# column permutation: old [z(400)|r(400)] -> new layout above, so r chunks are
# lane-aligned with h chunks (no on-chip realign DMAs)
ZR_PERM = np.zeros(800, dtype=np.int64)
for _g in range(2):
    ZR_PERM[_g * 384 : (_g + 1) * 384] = np.arange(_g * 400, _g * 400 + 384)
ZR_PERM[768:784] = np.arange(384, 400)        # z features 384..399
ZR_PERM[784:800] = np.arange(784, 800)        # r features 384..399
NB = 512                   # matmul free-dim tile (one PSUM bank of fp32)

AF = mybir.ActivationFunctionType
ALU = mybir.AluOpType

# packed bias column layout: name -> (offset, n_chunks)
BIAS_LAYOUT = {}
_off = 0
for _n, _c in (("bzr1", 7), ("bnx1", 4), ("bnh1", 4),
               ("bzr2", 7), ("bnx2", 4), ("bnh2", 4),
               ("bfc2", 5), ("bfc3", 5), ("bfc4", 3)):
    BIAS_LAYOUT[_n] = (_off, _c)
    _off += _c
BIAS_COLS = _off


def _pad2(a, rows, cols):
    out = np.zeros((rows, cols), dtype=np.float64)
    out[: a.shape[0], : a.shape[1]] = a
    return out


def _bias_tile(vec, padded):
    """Pack a [padded] bias vector as [128, padded//128] fp32 (partition-major)."""
    v = np.zeros(padded, dtype=np.float64)
    v[: vec.shape[0]] = vec
    return np.ascontiguousarray(v.reshape(padded // 128, 128).T).astype(np.float32)


def prepare_weights(inp):
    f64 = {k: np.asarray(v, dtype=np.float64) for k, v in inp.items()}
    w = {}

    # fc1 fold for GRU1 input side
    Wx = {}
    bx = {}
    for name in ("z", "r", "n"):
        Wx[name] = (f64[f"Wi{name}1"] @ f64["Wfc1"]).T          # [F, H]
        bx[name] = f64[f"bi{name}1"] + f64[f"Wi{name}1"] @ f64["bfc1"]

    # GRU1 z,r: K-concat [x(257) | h1(400)] -> rows 0..656, M=[z|r]=800
    Wzr1 = np.zeros((768, ZRM), dtype=np.float64)
    for g, name in enumerate(("z", "r")):
        Wzr1[:F, g * H : g * H + H] = Wx[name]
        Wzr1[F : F + H, g * H : g * H + H] = f64[f"Wh{name}1"].T
    w["Wzr1"] = Wzr1[:, ZR_PERM]
    # GRU1 n input side contracts xh chunks 0..3; rows 257.. are h1 -> zero
    w["Wn1x"] = _pad2(Wx["n"], Hp, Hp)
    w["Wn1h"] = _pad2(f64["Whn1"].T, Hp, Hp)

    # GRU2 z,r: two accumulation groups (g1-side, h2-side), each K=512 padded
    Wzr2g = np.zeros((Hp, ZRM), dtype=np.float64)
    Wzr2h = np.zeros((Hp, ZRM), dtype=np.float64)
    for g, name in enumerate(("z", "r")):
        Wzr2g[:H, g * H : g * H + H] = f64[f"Wi{name}2"].T
        Wzr2h[:H, g * H : g * H + H] = f64[f"Wh{name}2"].T
    w["Wzr2g"] = Wzr2g[:, ZR_PERM]
    w["Wzr2h"] = Wzr2h[:, ZR_PERM]
    w["Wn2x"] = _pad2(f64["Win2"].T, Hp, Hp)
    w["Wn2h"] = _pad2(f64["Whn2"].T, Hp, Hp)

    w["Wfc2T"] = _pad2(f64["Wfc2"].T, Hp, FFp)    # [512, 640]
    w["Wfc3T"] = _pad2(f64["Wfc3"].T, FFp, FFp)   # [640, 640]
    w["Wfc4T"] = _pad2(f64["Wfc4"].T, FFp, Fp)    # [640, 384]

    weights = {k: np.ascontiguousarray(v).astype(FP8) for k, v in w.items()}

    parts = [
        ("bzr1", _bias_tile(np.concatenate([bx["z"] + f64["bhz1"],
                                            bx["r"] + f64["bhr1"]])[ZR_PERM],
                            896)),
        ("bnx1", _bias_tile(bx["n"], Hp)),
        ("bnh1", _bias_tile(f64["bhn1"], Hp)),
        ("bzr2", _bias_tile(np.concatenate([f64["biz2"] + f64["bhz2"],
                                            f64["bir2"] + f64["bhr2"]])[ZR_PERM],
                            896)),
        ("bnx2", _bias_tile(f64["bin2"], Hp)),
        ("bnh2", _bias_tile(f64["bhn2"], Hp)),
        ("bfc2", _bias_tile(f64["bfc2"], FFp)),
        ("bfc3", _bias_tile(f64["bfc3"], FFp)),
        ("bfc4", _bias_tile(f64["bfc4"], Fp)),
    ]
    biases = {"biasT": np.concatenate([p[1] for p in parts], axis=1)}
    return weights, biases


def build_nc(nbt=BPC, nb=NB):
    """Build the per-core Bass program. nbt = per-core batch, nb = free-dim tile."""
    nc = bacc.Bacc("TRN2", target_bir_lowering=False, debug=False)
    bf = mybir.dt.bfloat16
    f32 = mybir.dt.float32
    f8 = mybir.dt.float8e4

    xh8 = nc.declare_dram_parameter("xh8", [XH1, nbt], f8, isOutput=False)
    h1T = nc.declare_dram_parameter("h1T", [Hp, nbt], bf, isOutput=False)
    h2T = nc.declare_dram_parameter("h2T", [Hp, nbt], bf, isOutput=False)
    h28 = nc.declare_dram_parameter("h28", [Hp, nbt], f8, isOutput=False)
    wd = {}
    for name, k, m in (
        ("Wzr1", 768, ZRM), ("Wn1x", Hp, Hp), ("Wn1h", Hp, Hp),
        ("Wzr2g", Hp, ZRM), ("Wzr2h", Hp, ZRM),
        ("Wn2x", Hp, Hp), ("Wn2h", Hp, Hp),
        ("Wfc2T", Hp, FFp), ("Wfc3T", FFp, FFp), ("Wfc4T", FFp, Fp),
    ):
        wd[name] = nc.declare_dram_parameter(name, [k, m], f8, isOutput=False)
    biasT_d = nc.declare_dram_parameter("biasT", [128, BIAS_COLS], f32, isOutput=False)
    outT = nc.declare_dram_parameter("outT", [Fp, nbt], bf, isOutput=True)

    n_tiles = nbt // nb
    HC = Hp // 128   # 4
    FC = FFp // 128  # 5
    DR = mybir.MatmulPerfMode.DoubleRow

    with tile.TileContext(nc) as tc:
        with (
            tc.tile_pool(name="wpool", bufs=1) as wpool,
            tc.tile_pool(name="bpool", bufs=1) as bpool,
            tc.tile_pool(name="io", bufs=3) as io,
            tc.tile_pool(name="inp", bufs=3) as inp,
            tc.tile_pool(name="act", bufs=3) as act,
            tc.tile_pool(name="psum", bufs=2, space="PSUM") as psum,
        ):
            # ACT-table warmup off the critical chain.
            warm = bpool.tile([128, 1], f32, tag="warm")
            nc.vector.memset(warm, 0.0)
            nc.scalar.activation(warm, warm, AF.Sigmoid)
            # PE HAM warmup: ~3.4us of dummy matmuls on zeroed SBUF so the PE
            # clock is at 8/8 when the first real matmul's operands land.
            wz8 = bpool.tile([128, 2, 128], f8, tag="wz8")
            nc.vector.memset(wz8, 0.0)
            pw = psum.tile([128, nb], f32, tag="ps_zr")
            for _ in range(32):
                nc.tensor.matmul(pw[:, 0:128], wz8[:, 0, :], wz8[:, 1, :],
                                 start=True, stop=True)

            W, BT = {}, {}

            def load_w(name, eng):
                dram = wd[name]
                k, m = dram.shape
                t = wpool.tile([128, k // 128, m], dram.dtype, tag=name)
                eng.dma_start(out=t, in_=dram.rearrange("(c p) m -> p c m", p=128))
                W[name] = t

            def load_bias():
                biasT = bpool.tile([128, BIAS_COLS], f32, tag="biasT")
                nc.scalar.dma_start(out=biasT, in_=biasT_d[:, :])
                for _n, (_o, _c) in BIAS_LAYOUT.items():
                    BT[_n] = biasT[:, _o : _o + _c]

            xh_zr = xh8[0:768, :].rearrange("(c p) n -> p c n", p=128)
            h1m_al = xh8[257 : 257 + Hp, :].rearrange("(c p) n -> p c n", p=128)
            h1_bl = h1T.rearrange("(c p) n -> p c n", p=128)
            h2_bl = h2T.rearrange("(c p) n -> p c n", p=128)
            h28_al = h28.rearrange("(c p) n -> p c n", p=128)
            outT_r = outT.rearrange("(c p) n -> p c n", p=128)

            INP, ZOP, G2 = {}, {}, {}

            def load_inputs(t):
                sl = ts(t, nb)
                xh = inp.tile([128, 6, nb], f8, tag="xh")
                nc.sync.dma_start(out=xh, in_=xh_zr[:, :, sl])
                h1m = inp.tile([128, HC, nb], f8, tag="h1m")
                nc.sync.dma_start(out=h1m, in_=h1m_al[:, :, sl])
                h1s = inp.tile([128, HC, nb], bf, tag="h1s")
                nc.sync.dma_start(out=h1s, in_=h1_bl[:, :, sl])
                h2s = inp.tile([128, HC, nb], bf, tag="h2s")
                nc.sync.dma_start(out=h2s, in_=h2_bl[:, :, sl])
                h28s = inp.tile([128, HC, nb], f8, tag="h28s")
                nc.sync.dma_start(out=h28s, in_=h28_al[:, :, sl])
                INP[t] = (xh, h1m, h1s, h2s, h28s)

            def mm_acc(ps, parts, col, mw):
                """Accumulate sum_g (W_g.T @ rhs_g) for output cols [col, col+mw)
                into ps. parts: list of (Wtile, kc, rhs_tile); DR on chunk pairs."""
                total = sum((kc + 1) // 2 for _, kc, _ in parts)
                i = 0
                for Wt, kc, rhs in parts:
                    k = 0
                    while k < kc:
                        if k + 2 <= kc:
                            nc.tensor.matmul(
                                ps, Wt[:, k : k + 2, col : col + mw],
                                rhs[:, k : k + 2, :],
                                start=(i == 0), stop=(i == total - 1),
                                perf_mode=DR)
                            k += 2
                        else:
                            nc.tensor.matmul(
                                ps, Wt[:, k, col : col + mw], rhs[:, k, :],
                                start=(i == 0), stop=(i == total - 1))
                            k += 1
                        i += 1

            def make_gru(zr_parts, nx_parts, nh_parts, hs, bzr, bnx, bnh,
                         out_chunk, pre_z0=None, add_eng=None):
                """Unit closures for one GRU step. h' = z*h + (1-z)*n.
                zro chunks: 0-2 = z[0:384], 3-5 = r[0:384] (lane-aligned with
                h chunks), 6 = [z-stub(16) | r-stub(16)]."""
                st = {}

                def z_unit(c):
                    def run():
                        if c == 0:
                            if pre_z0 is not None:
                                pre_z0()
                            st["zro"] = act.tile([128, ZRC, nb], bf, tag="zro", name="zro")
                        zro = st["zro"]
                        mw = min(128, ZRM - c * 128)
                        ps = psum.tile([128, nb], f32, tag="ps_zr")
                        mm_acc(ps[:mw, :], zr_parts, c * 128, mw)
                        nc.scalar.activation(zro[:mw, c, :], ps[:mw, :],
                                             AF.Sigmoid, bias=bzr[:mw, c : c + 1])
                        if c == 6:
                            # r stub: parts 16:32 of chunk 6 -> parts 0:16
                            r_st = act.tile([128, nb], bf, tag="r_st")
                            nc.sync.dma_start(out=r_st[0:16, :],
                                              in_=zro[16:32, 6, :])
                            st["r_st"] = r_st
                            zh = act.tile([128, HC, nb], bf, tag="zh")
                            u = act.tile([128, HC, nb], bf, tag="u")
                            nc.gpsimd.tensor_mul(zh[:, 0:3, :], zro[:, 0:3, :],
                                                 hs[:, 0:3, :])
                            nc.gpsimd.tensor_mul(zh[0:16, 3, :], zro[0:16, 6, :],
                                                 hs[0:16, 3, :])
                            nc.gpsimd.tensor_scalar(u[:, 0:3, :], zro[:, 0:3, :],
                                                    -1.0, 1.0, op0=ALU.mult,
                                                    op1=ALU.add)
                            nc.gpsimd.tensor_scalar(u[0:16, 3, :],
                                                    zro[0:16, 6, :], -1.0, 1.0,
                                                    op0=ALU.mult, op1=ALU.add)
                            st["zh"], st["u"] = zh, u
                    return run

                def n_unit(m):
                    def run():
                        zro, r_st = st["zro"], st["r_st"]
                        zh, u = st["zh"], st["u"]
                        pz = 128 if m < 3 else 16
                        col = m * 128
                        psh = psum.tile([128, nb], f32, tag="ps_nh")
                        mm_acc(psh, nh_parts, col, 128)
                        psx = psum.tile([128, nb], f32, tag="ps_nx")
                        mm_acc(psx, nx_parts, col, 128)
                        r_ap = zro[:pz, 3 + m, :] if m < 3 else r_st[0:16, :]
                        rhn = act.tile([128, nb], bf, tag="rhn")
                        nc.vector.scalar_tensor_tensor(
                            rhn[:pz, :], psh[:pz, :], bnh[:pz, m : m + 1],
                            r_ap, op0=ALU.add, op1=ALU.mult)
                        npre = act.tile([128, nb], f32, tag="npre")
                        nc.vector.scalar_tensor_tensor(
                            npre[:pz, :], psx[:pz, :], bnx[:pz, m : m + 1],
                            rhn[:pz, :], op0=ALU.add, op1=ALU.add)
                        n_t = act.tile([128, nb], bf, tag="n_t")
                        nc.scalar.activation(n_t[:pz, :], npre[:pz, :], AF.Tanh)
                        un = act.tile([128, nb], bf, tag="un")
                        nc.vector.tensor_mul(un[:pz, :], u[:pz, m, :], n_t[:pz, :])
                        out_ap = out_chunk(m)
                        p = min(out_ap.shape[0], pz)
                        (add_eng or nc.vector).tensor_add(
                            out_ap[:p, :] if out_ap.shape[0] > p else out_ap,
                            zh[:p, m, :], un[:p, :])
                    return run

                return [z_unit(c) for c in range(ZRC)], \
                       [n_unit(m) for m in range(HC)]

            def make_gru1(t):
                xh, h1m, h1s, h2s, h28s = INP[t]
                zop = io.tile([128, HC, nb], f8, tag="zop")
                ZOP[t] = zop

                def pre_z0():
                    nc.gpsimd.memset(zop[:, 3, :], 0.0)

                def g1_out(m):
                    return zop[:, m, :] if m < 3 else zop[0:16, 3, :]

                return make_gru([(W["Wzr1"], 6, xh)],
                                [(W["Wn1x"], HC, xh)],
                                [(W["Wn1h"], HC, h1m)],
                                h1s, BT["bzr1"], BT["bnx1"], BT["bnh1"],
                                g1_out, pre_z0)

            def make_gru2(t):
                xh, h1m, h1s, h2s, h28s = INP[t]
                zop = ZOP[t]
                g2 = io.tile([128, HC, nb], f8, tag="g2")
                G2[t] = g2

                def pre_z0():
                    nc.gpsimd.memset(g2[:, 3, :], 0.0)

                return make_gru([(W["Wzr2g"], HC, zop), (W["Wzr2h"], HC, h28s)],
                                [(W["Wn2x"], HC, zop)],
                                [(W["Wn2h"], HC, h28s)],
                                h2s, BT["bzr2"], BT["bnx2"], BT["bnh2"],
                                lambda m: g2[:, m, :] if m < 3 else g2[0:16, 3, :],
                                pre_z0)

            def make_fc(t):
                """Unit closures for fc2/fc3/fc4 of tile t. f3 alternates its
                PSUM between ps_fc and ps_nh so back-to-back groups never wait
                on the relu of the group two before."""
                g2 = G2[t]
                st = {}

                def f2_unit(m):
                    def run():
                        if m == 0:
                            st["f2"] = io.tile([128, FC, nb], f8, tag="f2", name="f2")
                        tag = "ps_fc" if m % 2 == 0 else "ps_zr"
                        ps = psum.tile([128, nb], f32, tag=tag)
                        mm_acc(ps, [(W["Wfc2T"], HC, g2)], m * 128, 128)
                        nc.scalar.activation(st["f2"][:, m, :], ps, AF.Relu,
                                             bias=BT["bfc2"][:, m : m + 1])
                    return run

                def f3_unit(m):
                    def run():
                        if m == 0:
                            st["f3"] = io.tile([128, FC, nb], f8, tag="f3", name="f3")
                        tag = "ps_fc" if m % 2 == 0 else "ps_nh"
                        ps = psum.tile([128, nb], f32, tag=tag)
                        mm_acc(ps, [(W["Wfc3T"], FC, st["f2"])], m * 128, 128)
                        nc.vector.tensor_scalar(st["f3"][:, m, :], ps,
                                                BT["bfc3"][:, m : m + 1], 0.0,
                                                op0=ALU.add, op1=ALU.max)
                    return run

                def f4_unit(m):
                    def run():
                        if m == 0:
                            st["o"] = io.tile([128, Fp // 128, nb], bf, tag="o", name="o")
                        ps = psum.tile([128, nb], f32, tag="ps_fc")
                        mm_acc(ps, [(W["Wfc4T"], FC, st["f3"])], m * 128, 128)
                        nc.scalar.activation(st["o"][:, m, :], ps, AF.Sigmoid,
                                             bias=BT["bfc4"][:, m : m + 1])
                    return run

                def out_unit():
                    nc.sync.dma_start(out=outT_r[:, :, ts(t, nb)], in_=st["o"])

                return ([f2_unit(m) for m in range(FC)],
                        [f3_unit(m) for m in range(FC)],
                        [f4_unit(m) for m in range(Fp // 128)], out_unit)

            # startup: interleave tile-0 inputs with the weights needed first
            sl0 = ts(0, nb)
            load_w("Wzr1", nc.sync)
            xh0 = inp.tile([128, 6, nb], f8, tag="xh")
            nc.sync.dma_start(out=xh0, in_=xh_zr[:, :, sl0])
            load_bias()
            h1m0 = inp.tile([128, HC, nb], f8, tag="h1m")
            nc.sync.dma_start(out=h1m0, in_=h1m_al[:, :, sl0])
            load_w("Wn1x", nc.scalar)
            load_w("Wn1h", nc.scalar)
            h1s0 = inp.tile([128, HC, nb], bf, tag="h1s")
            nc.sync.dma_start(out=h1s0, in_=h1_bl[:, :, sl0])
            h2s0 = inp.tile([128, HC, nb], bf, tag="h2s")
            nc.sync.dma_start(out=h2s0, in_=h2_bl[:, :, sl0])
            h28s0 = inp.tile([128, HC, nb], f8, tag="h28s")
            nc.sync.dma_start(out=h28s0, in_=h28_al[:, :, sl0])
            INP[0] = (xh0, h1m0, h1s0, h2s0, h28s0)
            for name in ("Wzr2g", "Wzr2h", "Wn2x", "Wn2h",
                         "Wfc2T", "Wfc3T", "Wfc4T"):
                load_w(name, nc.scalar)

            # Steady-state PE emission per iteration t (units for tiles
            # t-1 / t / t+1): GRU1(t)'s n units interleaved with FC(t-1)'s
            # fc2, then fc3/fc4, then GRU2(t)'s zr, then GRU2(t)'s n units
            # interleaved with GRU1(t+1)'s zr — so the tensor engine always
            # has independent work while each gate chain drains.
            Z1, N1 = make_gru1(0)
            for z in Z1:
                z()
            Z1_done = False  # True when tile t+1's zr units were pre-emitted
            for t in range(n_tiles):
                if t + 1 < n_tiles:
                    load_inputs(t + 1)
                F2, F3, F4, FOUT = make_fc(t - 1) if t >= 1 else \
                    ([None] * FC, [None] * FC, [None] * 3, None)

                def emit(u):
                    if u is not None:
                        u()

                if t == 0:
                    # no FC(t-1) filler on the first tile: interleave the next
                    # tile's zr units into the n-chain instead
                    Z1n, N1n = make_gru1(1)
                    N1[0]()
                    N1[1]()
                    Z1n[0]()
                    Z1n[1]()
                    N1[2]()
                    Z1n[2]()
                    Z1n[3]()
                    N1[3]()
                    Z1n[4]()
                    Z1n[5]()
                    Z1n[6]()
                    Z1_done = True
                else:
                    N1[0]()
                    N1[1]()
                    emit(F2[0])
                    N1[2]()
                    emit(F2[1])
                    emit(F2[2])
                    N1[3]()
                    emit(F2[3])
                    emit(F2[4])
                for u in F3:
                    emit(u)
                Z2, N2 = make_gru2(t)
                Z2[0]()
                emit(F4[0])
                Z2[1]()
                emit(F4[1])
                Z2[2]()
                emit(F4[2])
                emit(FOUT)
                Z2[3]()
                Z2[4]()
                Z2[5]()
                Z2[6]()
                if t + 1 < n_tiles:
                    if not Z1_done:
                        Z1n, N1n = make_gru1(t + 1)
                        N2[0]()
                        Z1n[0]()
                        Z1n[1]()
                        N2[1]()
                        Z1n[2]()
                        Z1n[3]()
                        N2[2]()
                        Z1n[4]()
                        Z1n[5]()
                        N2[3]()
                        Z1n[6]()
                    else:
                        for u in N2:
                            u()
                        Z1_done = False
                    N1 = N1n
                else:
                    for u in N2:
                        u()
            F2, F3, F4, FOUT = make_fc(n_tiles - 1)
            for u in F2 + F3 + F4:
                u()
            FOUT()

    nc.compile()
    return nc


def _shard_inputs(inp, weights, biases):
    x = np.asarray(inp["x"], dtype=np.float32)
    h1 = np.asarray(inp["h1"], dtype=np.float32)
    h2 = np.asarray(inp["h2"], dtype=np.float32)

    xh8 = np.zeros((NCORES, XH1, BPC), dtype=FP8)    # matmul operand [x|h1]
    h1T = np.zeros((NCORES, Hp, BPC), dtype=BF16)    # blend h1
    h2T = np.zeros((NCORES, Hp, BPC), dtype=BF16)    # blend h2
    h28 = np.zeros((NCORES, Hp, BPC), dtype=FP8)     # matmul h2
    for i in range(NCORES):
        sl = slice(i * BPC, (i + 1) * BPC)
        xh8[i, :F] = x[sl].T.astype(FP8)
        xh8[i, F : F + H] = h1[sl].T.astype(FP8)
        h1T[i, :H] = h1[sl].T.astype(BF16)
        h2T[i, :H] = h2[sl].T.astype(BF16)
        h28[i, :H] = h2[sl].T.astype(FP8)

    in_maps = []
    for i in range(NCORES):
        m = {"xh8": xh8[i], "h1T": h1T[i], "h2T": h2T[i], "h28": h28[i]}
        m.update(weights)
        m.update(biases)
        in_maps.append(m)
    return in_maps


def _run(inp, trace=False):
    weights, biases = prepare_weights(inp)
    nc = build_nc()
    in_maps = _shard_inputs(inp, weights, biases)
    res = run_bass_kernel_spmd(nc, in_maps, list(range(NCORES)), trace=trace)
    out = np.empty((B, F), dtype=np.float32)
    for i in range(NCORES):
        out[i * BPC : (i + 1) * BPC] = (
            np.asarray(res.results[i]["outT"][:F]).astype(np.float32).T
        )
    return out, res


def kernel(**inputs) -> np.ndarray:
    out, _ = _run(inputs, trace=False)
    return out


# revision 32
# speedup vs baseline: 1.0465x; 1.0254x over previous
"""NsNet2 single-step (fc1 + 2x GRU cell + 3x FC) Trainium2 kernel.

Strategy:
  - Pure data parallel: batch B=32768 sharded as 4096 rows per NeuronCore (8 cores).
  - Feature-major layout on chip: activations live as [feat, batch]; host
    transposes inputs/outputs (free; off the HW critical path).
  - All matmuls fp8(e4m3) with DoubleRow perf mode (2 K-chunks per pass) and
    fp32 PSUM accumulation; biases fused into ScalarE activation / VectorE ops.
  - fc1 folded into the GRU1 input-gate weights on the host.
  - z,r gates: input-side and hidden-side summed in one PSUM. GRU1 contracts a
    host-packed [x|h1] operand (657 rows -> 6 chunks, 3 DR passes); GRU2 uses
    two accumulation groups (g1-aligned + h2-aligned, 2+2 DR passes) to avoid
    an on-chip concat.
  - Software pipelining across the 8 batch tiles per core: PE work is emitted
    at matmul-group granularity as GRU1(t)'s n-units interleaved with FC(t-1),
    then GRU2(t)'s zr, then GRU2(t)'s n-units interleaved with GRU1(t+1)'s zr,
    so the tensor engine always has independent work while gate chains drain.
  - PE HAM warmup (dummy matmuls during the initial weight DMA) so real matmuls
    start at the full 2.4 GHz clock.
  - Blend h' = z*h + (1-z)*n: z*h and (1-z) precomputed on GpSimd off the
    critical path; only u*n and the final add (VectorE) trail the tanh.
"""

import os
import sys

import numpy as np
import ml_dtypes

sys.path.insert(0, "/opt/trn_rl_repo")

import concourse.bacc as bacc
import concourse.bass as bass
import concourse.mybir as mybir
import concourse.tile as tile
from concourse.bass import ts
from concourse.bass_utils import run_bass_kernel_spmd

BF16 = ml_dtypes.bfloat16
FP8 = ml_dtypes.float8_e4m3

B, F, H, FF = 32768, 257, 400, 600
NCORES = 8
BPC = B // NCORES          # 4096 batch rows per core
Hp, FFp, Fp = 512, 640, 384  # padded feature dims
XH1 = 769                  # [x(257) | h1(400) | pad(112)] rows
ZRM = 800                  # [z(0:384) | r(0:384) | z-stub(16) | r-stub(16)] cols
ZRC = 7                    # 6 full M chunks + 32-wide stub chunk
# column permutation: old [z(400)|r(400)] -> new layout above, so r chunks are
# lane-aligned with h chunks (no on-chip realign DMAs)
ZR_PERM = np.zeros(800, dtype=np.int64)
for _g in range(2):
    ZR_PERM[_g * 384 : (_g + 1) * 384] = np.arange(_g * 400, _g * 400 + 384)
ZR_PERM[768:784] = np.arange(384, 400)        # z features 384..399
ZR_PERM[784:800] = np.arange(784, 800)        # r features 384..399
NB = 512                   # matmul free-dim tile (one PSUM bank of fp32)

AF = mybir.ActivationFunctionType
ALU = mybir.AluOpType

# packed bias column layout: name -> (offset, n_chunks)
BIAS_LAYOUT = {}
_off = 0
for _n, _c in (("bzr1", 7), ("bnx1", 4), ("bnh1", 4),
               ("bzr2", 7), ("bnx2", 4), ("bnh2", 4),
               ("bfc2", 5), ("bfc3", 5), ("bfc4", 3)):
    BIAS_LAYOUT[_n] = (_off, _c)
    _off += _c
BIAS_COLS = _off


def _pad2(a, rows, cols):
    out = np.zeros((rows, cols), dtype=np.float64)
    out[: a.shape[0], : a.shape[1]] = a
    return out


def _bias_tile(vec, padded):
    """Pack a [padded] bias vector as [128, padded//128] fp32 (partition-major)."""
    v = np.zeros(padded, dtype=np.float64)
    v[: vec.shape[0]] = vec
    return np.ascontiguousarray(v.reshape(padded // 128, 128).T).astype(np.float32)


def prepare_weights(inp):
    f64 = {k: np.asarray(v, dtype=np.float64) for k, v in inp.items()}
    w = {}

    # fc1 fold for GRU1 input side
    Wx = {}
    bx = {}
    for name in ("z", "r", "n"):
        Wx[name] = (f64[f"Wi{name}1"] @ f64["Wfc1"]).T          # [F, H]
        bx[name] = f64[f"bi{name}1"] + f64[f"Wi{name}1"] @ f64["bfc1"]

    # GRU1 z,r: K-concat [x(257) | h1(400)] -> rows 0..656, M=[z|r]=800
    Wzr1 = np.zeros((768, ZRM), dtype=np.float64)
    for g, name in enumerate(("z", "r")):
        Wzr1[:F, g * H : g * H + H] = Wx[name]
        Wzr1[F : F + H, g * H : g * H + H] = f64[f"Wh{name}1"].T
    w["Wzr1"] = Wzr1[:, ZR_PERM]
    # GRU1 n input side contracts xh chunks 0..3; rows 257.. are h1 -> zero
    w["Wn1x"] = _pad2(Wx["n"], Hp, Hp)
    w["Wn1h"] = _pad2(f64["Whn1"].T, Hp, Hp)

    # GRU2 z,r: two accumulation groups (g1-side, h2-side), each K=512 padded
    Wzr2g = np.zeros((Hp, ZRM), dtype=np.float64)
    Wzr2h = np.zeros((Hp, ZRM), dtype=np.float64)
    for g, name in enumerate(("z", "r")):
        Wzr2g[:H, g * H : g * H + H] = f64[f"Wi{name}2"].T
        Wzr2h[:H, g * H : g * H + H] = f64[f"Wh{name}2"].T
    w["Wzr2g"] = Wzr2g[:, ZR_PERM]
    w["Wzr2h"] = Wzr2h[:, ZR_PERM]
    w["Wn2x"] = _pad2(f64["Win2"].T, Hp, Hp)
    w["Wn2h"] = _pad2(f64["Whn2"].T, Hp, Hp)

    w["Wfc2T"] = _pad2(f64["Wfc2"].T, Hp, FFp)    # [512, 640]
    w["Wfc3T"] = _pad2(f64["Wfc3"].T, FFp, FFp)   # [640, 640]
    w["Wfc4T"] = _pad2(f64["Wfc4"].T, FFp, Fp)    # [640, 384]

    weights = {k: np.ascontiguousarray(v).astype(FP8) for k, v in w.items()}

    parts = [
        ("bzr1", _bias_tile(np.concatenate([bx["z"] + f64["bhz1"],
                                            bx["r"] + f64["bhr1"]])[ZR_PERM],
                            896)),
        ("bnx1", _bias_tile(bx["n"], Hp)),
        ("bnh1", _bias_tile(f64["bhn1"], Hp)),
        ("bzr2", _bias_tile(np.concatenate([f64["biz2"] + f64["bhz2"],
                                            f64["bir2"] + f64["bhr2"]])[ZR_PERM],
                            896)),
        ("bnx2", _bias_tile(f64["bin2"], Hp)),
        ("bnh2", _bias_tile(f64["bhn2"], Hp)),
        ("bfc2", _bias_tile(f64["bfc2"], FFp)),
        ("bfc3", _bias_tile(f64["bfc3"], FFp)),
        ("bfc4", _bias_tile(f64["bfc4"], Fp)),
    ]
    biases = {"biasT": np.concatenate([p[1] for p in parts], axis=1)}
    return weights, biases


def build_nc(nbt=BPC, nb=NB):
    """Build the per-core Bass program. nbt = per-core batch, nb = free-dim tile."""
    nc = bacc.Bacc("TRN2", target_bir_lowering=False, debug=False)
    bf = mybir.dt.bfloat16
    f32 = mybir.dt.float32
    f8 = mybir.dt.float8e4

    xh8 = nc.declare_dram_parameter("xh8", [XH1, nbt], f8, isOutput=False)
    h1T = nc.declare_dram_parameter("h1T", [Hp, nbt], bf, isOutput=False)
    h2T = nc.declare_dram_parameter("h2T", [Hp, nbt], bf, isOutput=False)
    h28 = nc.declare_dram_parameter("h28", [Hp, nbt], f8, isOutput=False)
    wd = {}
    for name, k, m in (
        ("Wzr1", 768, ZRM), ("Wn1x", Hp, Hp), ("Wn1h", Hp, Hp),
        ("Wzr2g", Hp, ZRM), ("Wzr2h", Hp, ZRM),
        ("Wn2x", Hp, Hp), ("Wn2h", Hp, Hp),
        ("Wfc2T", Hp, FFp), ("Wfc3T", FFp, FFp), ("Wfc4T", FFp, Fp),
    ):
        wd[name] = nc.declare_dram_parameter(name, [k, m], f8, isOutput=False)
    biasT_d = nc.declare_dram_parameter("biasT", [128, BIAS_COLS], f32, isOutput=False)
    outT = nc.declare_dram_parameter("outT", [Fp, nbt], bf, isOutput=True)

    n_tiles = nbt // nb
    HC = Hp // 128   # 4
    FC = FFp // 128  # 5
    DR = mybir.MatmulPerfMode.DoubleRow

    with tile.TileContext(nc) as tc:
        with (
            tc.tile_pool(name="wpool", bufs=1) as wpool,
            tc.tile_pool(name="bpool", bufs=1) as bpool,
            tc.tile_pool(name="io", bufs=3) as io,
            tc.tile_pool(name="inp", bufs=3) as inp,
            tc.tile_pool(name="act", bufs=3) as act,
            tc.tile_pool(name="psum", bufs=2, space="PSUM") as psum,
        ):
            # ACT-table warmup off the critical chain.
            warm = bpool.tile([128, 1], f32, tag="warm")
            nc.vector.memset(warm, 0.0)
            nc.scalar.activation(warm, warm, AF.Sigmoid)
            # PE HAM warmup: ~3.4us of dummy matmuls on zeroed SBUF so the PE
            # clock is at 8/8 when the first real matmul's operands land.
            wz8 = bpool.tile([128, 2, 128], f8, tag="wz8")
            nc.vector.memset(wz8, 0.0)
            pw = psum.tile([128, nb], f32, tag="ps_zr")
            for _ in range(32):
                nc.tensor.matmul(pw[:, 0:128], wz8[:, 0, :], wz8[:, 1, :],
                                 start=True, stop=True)

            W, BT = {}, {}

            def load_w(name, eng):
                dram = wd[name]
                k, m = dram.shape
                t = wpool.tile([128, k // 128, m], dram.dtype, tag=name)
                eng.dma_start(out=t, in_=dram.rearrange("(c p) m -> p c m", p=128))
                W[name] = t

            def load_bias():
                biasT = bpool.tile([128, BIAS_COLS], f32, tag="biasT")
                nc.scalar.dma_start(out=biasT, in_=biasT_d[:, :])
                for _n, (_o, _c) in BIAS_LAYOUT.items():
                    BT[_n] = biasT[:, _o : _o + _c]

            xh_zr = xh8[0:768, :].rearrange("(c p) n -> p c n", p=128)
            h1m_al = xh8[257 : 257 + Hp, :].rearrange("(c p) n -> p c n", p=128)
            h1_bl = h1T.rearrange("(c p) n -> p c n", p=128)
            h2_bl = h2T.rearrange("(c p) n -> p c n", p=128)
            h28_al = h28.rearrange("(c p) n -> p c n", p=128)
            outT_r = outT.rearrange("(c p) n -> p c n", p=128)

            INP, ZOP, G2 = {}, {}, {}

            def load_inputs(t):
                sl = ts(t, nb)
                xh = inp.tile([128, 6, nb], f8, tag="xh")
                nc.sync.dma_start(out=xh, in_=xh_zr[:, :, sl])
                h1m = inp.tile([128, HC, nb], f8, tag="h1m")
                nc.sync.dma_start(out=h1m, in_=h1m_al[:, :, sl])
                h1s = inp.tile([128, HC, nb], bf, tag="h1s")
                nc.sync.dma_start(out=h1s, in_=h1_bl[:, :, sl])
                h2s = inp.tile([128, HC, nb], bf, tag="h2s")
                nc.sync.dma_start(out=h2s, in_=h2_bl[:, :, sl])
                h28s = inp.tile([128, HC, nb], f8, tag="h28s")
                nc.sync.dma_start(out=h28s, in_=h28_al[:, :, sl])
                INP[t] = (xh, h1m, h1s, h2s, h28s)

            def mm_acc(ps, parts, col, mw):
                """Accumulate sum_g (W_g.T @ rhs_g) for output cols [col, col+mw)
                into ps. parts: list of (Wtile, kc, rhs_tile); DR on chunk pairs."""
                total = sum((kc + 1) // 2 for _, kc, _ in parts)
                i = 0
                for Wt, kc, rhs in parts:
                    k = 0
                    while k < kc:
                        if k + 2 <= kc:
                            nc.tensor.matmul(
                                ps, Wt[:, k : k + 2, col : col + mw],
                                rhs[:, k : k + 2, :],
                                start=(i == 0), stop=(i == total - 1),
                                perf_mode=DR)
                            k += 2
                        else:
                            nc.tensor.matmul(
                                ps, Wt[:, k, col : col + mw], rhs[:, k, :],
                                start=(i == 0), stop=(i == total - 1))
                            k += 1
                        i += 1

            def make_gru(zr_parts, nx_parts, nh_parts, hs, bzr, bnx, bnh,
                         out_chunk, pre_z0=None, add_eng=None):
                """Unit closures for one GRU step. h' = z*h + (1-z)*n.
                zro chunks: 0-2 = z[0:384], 3-5 = r[0:384] (lane-aligned with
                h chunks), 6 = [z-stub(16) | r-stub(16)]."""
                st = {}

                def z_unit(c):
                    def run():
                        if c == 0:
                            if pre_z0 is not None:
                                pre_z0()
                            st["zro"] = act.tile([128, ZRC, nb], bf, tag="zro", name="zro")
                        zro = st["zro"]
                        mw = min(128, ZRM - c * 128)
                        ps = psum.tile([128, nb], f32, tag="ps_zr")
                        mm_acc(ps[:mw, :], zr_parts, c * 128, mw)
                        nc.scalar.activation(zro[:mw, c, :], ps[:mw, :],
                                             AF.Sigmoid, bias=bzr[:mw, c : c + 1])
                        if c == 6:
                            # r stub: parts 16:32 of chunk 6 -> parts 0:16
                            r_st = act.tile([128, nb], bf, tag="r_st")
                            nc.sync.dma_start(out=r_st[0:16, :],
                                              in_=zro[16:32, 6, :])
                            st["r_st"] = r_st
                            zh = act.tile([128, HC, nb], bf, tag="zh")
                            u = act.tile([128, HC, nb], bf, tag="u")
                            nc.gpsimd.tensor_mul(zh[:, 0:3, :], zro[:, 0:3, :],
                                                 hs[:, 0:3, :])
                            nc.gpsimd.tensor_mul(zh[0:16, 3, :], zro[0:16, 6, :],
                                                 hs[0:16, 3, :])
                            nc.gpsimd.tensor_scalar(u[:, 0:3, :], zro[:, 0:3, :],
                                                    -1.0, 1.0, op0=ALU.mult,
                                                    op1=ALU.add)
                            nc.gpsimd.tensor_scalar(u[0:16, 3, :],
                                                    zro[0:16, 6, :], -1.0, 1.0,
                                                    op0=ALU.mult, op1=ALU.add)
                            st["zh"], st["u"] = zh, u
                    return run

                def n_unit(m):
                    def run():
                        zro, r_st = st["zro"], st["r_st"]
                        zh, u = st["zh"], st["u"]
                        pz = 128 if m < 3 else 16
                        col = m * 128
                        psh = psum.tile([128, nb], f32, tag="ps_nh")
                        mm_acc(psh, nh_parts, col, 128)
                        psx = psum.tile([128, nb], f32, tag="ps_nx")
                        mm_acc(psx, nx_parts, col, 128)
                        r_ap = zro[:pz, 3 + m, :] if m < 3 else r_st[0:16, :]
                        rhn = act.tile([128, nb], bf, tag="rhn")
                        nc.vector.scalar_tensor_tensor(
                            rhn[:pz, :], psh[:pz, :], bnh[:pz, m : m + 1],
                            r_ap, op0=ALU.add, op1=ALU.mult)
                        npre = act.tile([128, nb], f32, tag="npre")
                        nc.vector.scalar_tensor_tensor(
                            npre[:pz, :], psx[:pz, :], bnx[:pz, m : m + 1],
                            rhn[:pz, :], op0=ALU.add, op1=ALU.add)
                        n_t = act.tile([128, nb], bf, tag="n_t")
                        nc.scalar.activation(n_t[:pz, :], npre[:pz, :], AF.Tanh)
                        un = act.tile([128, nb], bf, tag="un")
                        nc.vector.tensor_mul(un[:pz, :], u[:pz, m, :], n_t[:pz, :])
                        out_ap = out_chunk(m)
                        p = min(out_ap.shape[0], pz)
                        (add_eng or nc.vector).tensor_add(
                            out_ap[:p, :] if out_ap.shape[0] > p else out_ap,
                            zh[:p, m, :], un[:p, :])
                    return run

                return [z_unit(c) for c in range(ZRC)], \
                       [n_unit(m) for m in range(HC)]

            def make_gru1(t):
                xh, h1m, h1s, h2s, h28s = INP[t]
                zop = io.tile([128, HC, nb], f8, tag="zop")
                ZOP[t] = zop

                def pre_z0():
                    nc.gpsimd.memset(zop[:, 3, :], 0.0)

                def g1_out(m):
                    return zop[:, m, :] if m < 3 else zop[0:16, 3, :]

                return make_gru([(W["Wzr1"], 6, xh)],
                                [(W["Wn1x"], HC, xh)],
                                [(W["Wn1h"], HC, h1m)],
                                h1s, BT["bzr1"], BT["bnx1"], BT["bnh1"],
                                g1_out, pre_z0)

            def make_gru2(t):
                xh, h1m, h1s, h2s, h28s = INP[t]
                zop = ZOP[t]
                g2 = io.tile([128, HC, nb], f8, tag="g2")
                G2[t] = g2

                def pre_z0():
                    nc.gpsimd.memset(g2[:, 3, :], 0.0)

                return make_gru([(W["Wzr2g"], HC, zop), (W["Wzr2h"], HC, h28s)],
                                [(W["Wn2x"], HC, zop)],
                                [(W["Wn2h"], HC, h28s)],
                                h2s, BT["bzr2"], BT["bnx2"], BT["bnh2"],
                                lambda m: g2[:, m, :] if m < 3 else g2[0:16, 3, :],
                                pre_z0)

            def make_fc(t, split=False):
                """Unit closures for fc2/fc3/fc4 of tile t. f3/f2 alternate
                their PSUM tags so back-to-back groups never wait on the relu
                of the group two before. split=True (epilogue only) alternates
                each relu stage across Scalar+Vector so the final tile's
                serial drain halves."""
                g2 = G2[t]
                st = {}

                def f2_unit(m):
                    def run():
                        if m == 0:
                            st["f2"] = io.tile([128, FC, nb], f8, tag="f2", name="f2")
                        tag = "ps_fc" if m % 2 == 0 else "ps_zr"
                        ps = psum.tile([128, nb], f32, tag=tag)
                        mm_acc(ps, [(W["Wfc2T"], HC, g2)], m * 128, 128)
                        if split and m % 2 == 1:
                            nc.vector.tensor_scalar(st["f2"][:, m, :], ps,
                                                    BT["bfc2"][:, m : m + 1], 0.0,
                                                    op0=ALU.add, op1=ALU.max)
                        else:
                            nc.scalar.activation(st["f2"][:, m, :], ps, AF.Relu,
                                                 bias=BT["bfc2"][:, m : m + 1])
                    return run

                def f3_unit(m):
                    def run():
                        if m == 0:
                            st["f3"] = io.tile([128, FC, nb], f8, tag="f3", name="f3")
                        tag = "ps_fc" if m % 2 == 0 else "ps_nh"
                        ps = psum.tile([128, nb], f32, tag=tag)
                        mm_acc(ps, [(W["Wfc3T"], FC, st["f2"])], m * 128, 128)
                        if split and m % 2 == 1:
                            nc.scalar.activation(st["f3"][:, m, :], ps, AF.Relu,
                                                 bias=BT["bfc3"][:, m : m + 1])
                        else:
                            nc.vector.tensor_scalar(st["f3"][:, m, :], ps,
                                                    BT["bfc3"][:, m : m + 1], 0.0,
                                                    op0=ALU.add, op1=ALU.max)
                    return run

                def f4_unit(m):
                    def run():
                        if m == 0:
                            st["o"] = io.tile([128, Fp // 128, nb], bf, tag="o", name="o")
                        ps = psum.tile([128, nb], f32, tag="ps_fc")
                        mm_acc(ps, [(W["Wfc4T"], FC, st["f3"])], m * 128, 128)
                        nc.scalar.activation(st["o"][:, m, :], ps, AF.Sigmoid,
                                             bias=BT["bfc4"][:, m : m + 1])
                        # stream each output chunk as soon as its sigmoid lands
                        nc.sync.dma_start(out=outT_r[:, m, ts(t, nb)],
                                          in_=st["o"][:, m, :])
                    return run

                return ([f2_unit(m) for m in range(FC)],
                        [f3_unit(m) for m in range(FC)],
                        [f4_unit(m) for m in range(Fp // 128)])

            # startup: interleave tile-0 inputs with the weights needed first
            sl0 = ts(0, nb)
            load_w("Wzr1", nc.sync)
            xh0 = inp.tile([128, 6, nb], f8, tag="xh")
            nc.sync.dma_start(out=xh0, in_=xh_zr[:, :, sl0])
            load_bias()
            h1m0 = inp.tile([128, HC, nb], f8, tag="h1m")
            nc.sync.dma_start(out=h1m0, in_=h1m_al[:, :, sl0])
            load_w("Wn1x", nc.scalar)
            load_w("Wn1h", nc.scalar)
            h1s0 = inp.tile([128, HC, nb], bf, tag="h1s")
            nc.sync.dma_start(out=h1s0, in_=h1_bl[:, :, sl0])
            h2s0 = inp.tile([128, HC, nb], bf, tag="h2s")
            nc.sync.dma_start(out=h2s0, in_=h2_bl[:, :, sl0])
            h28s0 = inp.tile([128, HC, nb], f8, tag="h28s")
            nc.sync.dma_start(out=h28s0, in_=h28_al[:, :, sl0])
            INP[0] = (xh0, h1m0, h1s0, h2s0, h28s0)
            for name in ("Wzr2g", "Wzr2h", "Wn2x", "Wn2h",
                         "Wfc2T", "Wfc3T", "Wfc4T"):
                load_w(name, nc.scalar)

            # Steady-state PE emission per iteration t (units for tiles
            # t-1 / t / t+1): GRU1(t)'s n units interleaved with FC(t-1)'s
            # fc2, then fc3/fc4, then GRU2(t)'s zr, then GRU2(t)'s n units
            # interleaved with GRU1(t+1)'s zr — so the tensor engine always
            # has independent work while each gate chain drains.
            Z1, N1 = make_gru1(0)
            for z in Z1:
                z()
            Z1_done = False  # True when tile t+1's zr units were pre-emitted
            for t in range(n_tiles):
                if t + 1 < n_tiles:
                    load_inputs(t + 1)
                F2, F3, F4 = make_fc(t - 1) if t >= 1 else \
                    ([None] * FC, [None] * FC, [None] * 3)

                def emit(u):
                    if u is not None:
                        u()

                if t == 0:
                    # no FC(t-1) filler on the first tile: interleave the next
                    # tile's zr units into the n-chain instead
                    Z1n, N1n = make_gru1(1)
                    N1[0]()
                    N1[1]()
                    Z1n[0]()
                    Z1n[1]()
                    N1[2]()
                    Z1n[2]()
                    Z1n[3]()
                    N1[3]()
                    Z1n[4]()
                    Z1n[5]()
                    Z1n[6]()
                    Z1_done = True
                else:
                    N1[0]()
                    N1[1]()
                    emit(F2[0])
                    N1[2]()
                    emit(F2[1])
                    emit(F2[2])
                    N1[3]()
                    emit(F2[3])
                    emit(F2[4])
                for u in F3:
                    emit(u)
                Z2, N2 = make_gru2(t)
                Z2[0]()
                emit(F4[0])
                Z2[1]()
                emit(F4[1])
                Z2[2]()
                emit(F4[2])
                Z2[3]()
                Z2[4]()
                Z2[5]()
                Z2[6]()
                if t + 1 < n_tiles:
                    if not Z1_done:
                        Z1n, N1n = make_gru1(t + 1)
                        N2[0]()
                        Z1n[0]()
                        Z1n[1]()
                        N2[1]()
                        Z1n[2]()
                        Z1n[3]()
                        N2[2]()
                        Z1n[4]()
                        Z1n[5]()
                        N2[3]()
                        Z1n[6]()
                    else:
                        for u in N2:
                            u()
                        Z1_done = False
                    N1 = N1n
                else:
                    for u in N2:
                        u()
            F2, F3, F4 = make_fc(n_tiles - 1, split=True)
            for u in F2 + F3 + F4:
                u()

    nc.compile()
    return nc


def _shard_inputs(inp, weights, biases):
    x = np.asarray(inp["x"], dtype=np.float32)
    h1 = np.asarray(inp["h1"], dtype=np.float32)
    h2 = np.asarray(inp["h2"], dtype=np.float32)

    xh8 = np.zeros((NCORES, XH1, BPC), dtype=FP8)    # matmul operand [x|h1]
    h1T = np.zeros((NCORES, Hp, BPC), dtype=BF16)    # blend h1
    h2T = np.zeros((NCORES, Hp, BPC), dtype=BF16)    # blend h2
    h28 = np.zeros((NCORES, Hp, BPC), dtype=FP8)     # matmul h2
    for i in range(NCORES):
        sl = slice(i * BPC, (i + 1) * BPC)
        xh8[i, :F] = x[sl].T.astype(FP8)
        xh8[i, F : F + H] = h1[sl].T.astype(FP8)
        h1T[i, :H] = h1[sl].T.astype(BF16)
        h2T[i, :H] = h2[sl].T.astype(BF16)
        h28[i, :H] = h2[sl].T.astype(FP8)

    in_maps = []
    for i in range(NCORES):
        m = {"xh8": xh8[i], "h1T": h1T[i], "h2T": h2T[i], "h28": h28[i]}
        m.update(weights)
        m.update(biases)
        in_maps.append(m)
    return in_maps


def _run(inp, trace=False):
    weights, biases = prepare_weights(inp)
    nc = build_nc()
    in_maps = _shard_inputs(inp, weights, biases)
    res = run_bass_kernel_spmd(nc, in_maps, list(range(NCORES)), trace=trace)
    out = np.empty((B, F), dtype=np.float32)
    for i in range(NCORES):
        out[i * BPC : (i + 1) * BPC] = (
            np.asarray(res.results[i]["outT"][:F]).astype(np.float32).T
        )
    return out, res


def kernel(**inputs) -> np.ndarray:
    out, _ = _run(inputs, trace=False)
    return out
